# revision 61
# baseline (speedup 1.0000x reference)
"""BertSelfAttention Trainium2 Bass kernel (v2).

Full inputs in, full output out. Sharding: 8 cores = 4 batches x 2 head
groups (8 heads each). Per-core SPMD program (no collectives).

v2 structure (vs the v1 fp16 baseline at 301us):
  - Projections run in fp8 DoubleRow (cost-model 0.5 cyc/row, contraction
    256/instr): Q = (x8 + x8r) @ (w8 + w8r) dropping the rr term. Main
    tensors e4m3 (W host-prescaled x16 out of the subnormal range),
    residuals e5m2 (wide exponent range); all three terms share one PSUM
    scale so a single drain rescales by 1/16. X^T is host-transposed into
    the DoubleRow layout, removing all on-device transposes.
  - Scores and ctx stay fp16 (fp8 noise there lands ~3-6% in the final
    metric - over the 2e-2 gate). PE ~226us is the design bottleneck.
  - exp is split: each slot's two score matmuls write separate [128,512]
    PSUM tiles (ring of 4); per half-tile a fixed 32-period pattern
    assigns ACT (exact exp, 75%) or a DVE 2-term Schraudolph (25%):
    ia = trunc(score*A + B) int16 straight from PSUM, ib = ia - 512,
    pt = f16bits(ia) * f16bits(ib); the pair product cancels most of
    the linear-interp error (rms ~0.5%). Strict A/S interleaving keeps
    the exp engines off each other's ring slots. (GPSIMD cannot touch
    PSUM on real hw, so Pool only gets SBUF-side work: obs normalize,
    em scaling, memsets.)
  - ctx accumulated as in v1: [128 q, 65] chains packed into 2 PSUM
    banks, trailing cursor on a 16-deep pt ring; projection/boundary
    drains are emitted ~2 slots late so they never head-block the
    in-order DVE queue on a still-running PE chain.
"""

import sys
from contextlib import ExitStack

import numpy as np

sys.path.insert(0, "/opt/trn_rl_repo")

import concourse.bass as bass  # noqa: E402
from concourse import bacc  # noqa: E402
import concourse.mybir as mybir  # noqa: E402
import concourse.tile as tile  # noqa: E402

B, S, H = 4, 2048, 1024
NH, HD = 16, 64
GH = 8            # heads per core
GC = GH * HD      # 512 output cols per core
NP = 128          # partitions
NQ = S // 512     # 4 q blocks of 512
NKT = S // NP     # 16 k tiles of 128
NST = 16
F32 = mybir.dt.float32
F16 = mybir.dt.float16
I16 = mybir.dt.int16
F8E4 = mybir.dt.float8e4
F8E5 = mybir.dt.float8e5
SCALE = 1.0 / 8.0  # 1/sqrt(HD)
EXPF = mybir.ActivationFunctionType.Exp
DR = mybir.MatmulPerfMode.DoubleRow

# 2-term Schraudolph constants: ia = trunc(score * A2S + B2) (int16),
# exp(score/8) ~= f16bits(ia) * f16bits(ia - 512).
import math as _math
A2S = float(SCALE * 512.0 / _math.log(2.0))
B2 = 15561.25
WSCL = 1.0 / 16.0  # W host-prescale compensation in drains

import os as _os
N_WARM = int(_os.environ.get("K_NWARM", 11))
PT_RING = int(_os.environ.get("K_PTRING", 16))
CTX_LAG = int(_os.environ.get("K_CTXLAG", 14))
DRIP_NS = int(_os.environ.get("K_DRIP", 250))
BND_GATE = int(_os.environ.get("K_BGATE", 3))
# per-half exp engine pattern, period 32 over half-index (2*pos + i):
# True = DVE 2-term Schraudolph (8/32), False = exact ACT exp. Stride-4
# placement on odd halves keeps ACT/DVE strictly interleaved on the
# 4-deep PSUM score ring.
_S32S = {
    "8": {1, 5, 9, 13, 17, 21, 25, 29},
    "9o": {1, 3, 5, 9, 13, 17, 21, 25, 29},
    "9b": {1, 5, 9, 11, 15, 19, 23, 27, 31},
    "8p3": {3, 7, 11, 15, 19, 23, 27, 31},
    "10o": {1, 3, 7, 11, 13, 17, 21, 23, 27, 31},
    "9c": {1, 5, 7, 11, 15, 19, 23, 27, 31},
    "9d": {1, 5, 9, 13, 15, 19, 23, 27, 31},
    "9e": {3, 5, 9, 13, 17, 19, 23, 27, 31},
    "9f": {1, 5, 9, 13, 15, 19, 23, 27, 29},
    "9g": {1, 3, 7, 11, 15, 19, 23, 27, 31},
    "9h": {1, 5, 9, 13, 15, 17, 21, 25, 29},
}
_S32 = _S32S[_os.environ.get("K_S32", "9d")]
SCH_PAT = tuple(i in _S32 for i in range(32))


def _emit(tc, x8, x8r, wq8, wq8r, wk8, wk8r, wv8, wv8r, bqd, bkd, bvd,
          maskd, out):
    import heapq

    nc = tc.nc
    with ExitStack() as ctx:
        const = ctx.enter_context(tc.tile_pool(name="const", bufs=1))
        big = ctx.enter_context(tc.tile_pool(name="big", bufs=1))

        # ---- consts ----
        mask_sb = const.tile([NP, NKT], F32, tag="mask")
        em = const.tile([NP, NKT], F32, tag="em")
        bk_sb = const.tile([NP, 4], F32, tag="bk")
        bq_sb = const.tile([NP, 4], F32, tag="bq")
        bv_bc = const.tile([NP, GC], F32, tag="bvbc")

        # persistent SBUF tensors
        x8_sb = big.tile([NP, 16384], F8E4, tag="x8", name="x8")
        x8r_sb = big.tile([NP, 16384], F8E5, tag="x8r", name="x8r")
        qt_sb = [big.tile([NP, S], F16, tag=f"qt{i}", name=f"qt{i}") for i in range(4)]
        kt_sb = [big.tile([NP, S], F16, tag=f"kt{i}", name=f"kt{i}") for i in range(4)]
        v_sb = [
            [
                big.tile([NP, 2 * (HD + 1)], F16, tag=f"v{p}_{i}", name=f"v{p}_{i}")
                for i in range(NST)
            ]
            for p in range(4)
        ]
        wq_sb = big.tile([NP, 4096], F8E4, tag="wq")
        wqr_sb = big.tile([NP, 4096], F8E5, tag="wqr")
        wk_sb = big.tile([NP, 4096], F8E4, tag="wk")
        wkr_sb = big.tile([NP, 4096], F8E5, tag="wkr")
        wv_sb = big.tile([NP, 4096], F8E4, tag="wv")
        wvr_sb = big.tile([NP, 4096], F8E5, tag="wvr")

        def ld(dst, src, c0, cw):
            nc.sync.dma_start(out=dst[:, c0:c0 + cw], in_=src[:, c0:c0 + cw])

        # ---- DMA queue, ordered by first use; each DMACopy holds SP.SEQ
        # ~650ns, so the fat pr0-chain inputs issue FIRST and the tiny
        # bias/mask transfers (needed only at drain time ~5.5us) follow ----
        ld(wk_sb, wk8, 0, 1024)    # mt0 slices first: pr0 K/Q chains
        ld(wq_sb, wq8, 0, 1024)
        ld(x8_sb, x8, 0, 4096)     # s-block 0
        ld(wkr_sb, wk8r, 0, 1024)
        ld(wqr_sb, wq8r, 0, 1024)
        ld(x8r_sb, x8r, 0, 4096)
        nc.sync.dma_start(out=bk_sb[:], in_=bkd.rearrange("(m p) -> p m", p=NP))
        nc.sync.dma_start(out=bq_sb[:], in_=bqd.rearrange("(m p) -> p m", p=NP))
        nc.sync.dma_start(out=mask_sb[:], in_=maskd.rearrange("(t p) -> p t", p=NP))
        nc.scalar.activation(em[:], mask_sb[:], EXPF)  # warms Exp table too
        for sb in range(1, 4):     # s-blocks 1-3 (scores kt=4sb need them)
            ld(x8_sb, x8, sb * 4096, 4096)
            ld(x8r_sb, x8r, sb * 4096, 4096)
        ld(wv_sb, wv8, 0, 4096)
        ld(wvr_sb, wv8r, 0, 4096)
        nc.sync.dma_start(out=bv_bc[:], in_=bvd)
        ld(wk_sb, wk8, 1024, 3072)
        ld(wq_sb, wq8, 1024, 3072)
        ld(wkr_sb, wk8r, 1024, 3072)
        ld(wqr_sb, wq8r, 1024, 3072)

        def w_ap(w, mt, j):
            base = (mt * 4 + j) * 256
            return w[:, base:base + 256].rearrange("p (i m) -> p i m", i=2)

        def x_ap(xsb, sblk, j):
            base = (sblk * 4 + j) * 1024
            return xsb[:, base:base + 1024].rearrange("p (i s) -> p i s", i=2)

        with (
            tc.tile_pool(name="psS", bufs=4, space="PSUM") as psS,
            tc.tile_pool(name="psC", bufs=1, space="PSUM") as psC,
            tc.tile_pool(name="psB", bufs=2, space="PSUM") as psB,
            tc.tile_pool(name="ptpool", bufs=PT_RING) as ptpool,
            tc.tile_pool(name="iapool", bufs=3) as iapool,
            tc.tile_pool(name="ibpool", bufs=3) as ibpool,
            tc.tile_pool(name="cspool", bufs=4) as cspool,
            tc.tile_pool(name="rcpool", bufs=4) as rcpool,
            tc.tile_pool(name="obpool", bufs=1) as obpool,
        ):
            ctxps = [
                psC.tile([NP, 512], F32, tag=f"ctx{i}", name=f"ctx{i}")
                for i in range(2)
            ]
            obs = {
                qt: obpool.tile([NP, 4 * GC], F16, tag=f"ob{qt}", name="ob")
                for qt in range(NQ)
            }

            v_done = set()
            CUR = [0]       # current stream slot (for deferred drains)

            # terms: (w8,x8), (w8r,x8), (w8,x8r) - x8r arrives last via DMA
            _terms = ((0, 0), (1, 0), (0, 1))

            def v_chain(pr, st):
                """Project V cols for head pair pr, s-tile st (fp8 3-term)."""
                cell = {}
                sblk, sw = divmod(st, 4)

                def mm(t, j, cell=cell):
                    if t == 0 and j == 0:
                        cell["pv"] = psB.tile(
                            [NP, 512], F32, tag="proj", name="pv"
                        )[:, 0:NP]
                    wt, xt_ = _terms[t]
                    w = (wv_sb, wvr_sb)[wt]
                    xx = (x8_sb, x8r_sb)[xt_]
                    nc.tensor.matmul(
                        cell["pv"],
                        x_ap(xx, sblk, j)[:, :, sw * NP:(sw + 1) * NP],
                        w_ap(w, pr, j),
                        start=(t == 0 and j == 0),
                        stop=(t == 2 and j == 3),
                        perf_mode=DR,
                    )

                def drain(cell=cell):
                    v3 = v_sb[pr][st][:].rearrange("p (h e) -> p h e", e=HD + 1)
                    nc.gpsimd.memset(v3[:, :, HD], 1.0)
                    nc.vector.scalar_tensor_tensor(
                        out=v3[:, :, 0:HD],
                        in0=cell["pv"].rearrange("p (h e) -> p h e", e=HD),
                        scalar=WSCL,
                        in1=bv_bc[:, pr * NP:(pr + 1) * NP].rearrange(
                            "p (h e) -> p h e", e=HD
                        ),
                        op0=mybir.AluOpType.mult,
                        op1=mybir.AluOpType.add,
                    )
                    nc.gpsimd.tensor_scalar_mul(
                        v_sb[pr][st][:], v_sb[pr][st][:], em[:, st:st + 1]
                    )
                    v_done.add((pr, st))

                def lastmm(mm=mm, drain=drain):
                    # defer the drain ~2 slots so it never head-blocks the
                    # DVE queue waiting on this chain's PE completion
                    mm(2, 3)
                    push(CUR[0] + 2, [(0, drain)])

                return [
                    (27, lambda t=t, j=j, mm=mm: mm(t, j))
                    for t in range(3) for j in range(4)
                ][:-1] + [(27, lastmm)]

            def qk_chain(mt, which, nt, defer=True):
                wm, wr, dst, bias = (
                    (wk_sb, wkr_sb, kt_sb, bk_sb),
                    (wq_sb, wqr_sb, qt_sb, bq_sb),
                )[which]
                cell = {}

                def mm(t, j, cell=cell):
                    if t == 0 and j == 0:
                        cell["pp"] = psB.tile([NP, 512], F32, tag="proj", name="pp")
                    wt, xt_ = _terms[t]
                    w = (wm, wr)[wt]
                    xx = (x8_sb, x8r_sb)[xt_]
                    nc.tensor.matmul(
                        cell["pp"][:],
                        w_ap(w, mt, j),
                        x_ap(xx, nt, j),
                        start=(t == 0 and j == 0),
                        stop=(t == 2 and j == 3),
                        perf_mode=DR,
                    )

                def drain(cell=cell):
                    nc.vector.tensor_scalar(
                        out=dst[mt][:, nt * 512:(nt + 1) * 512],
                        in0=cell["pp"][:],
                        scalar1=WSCL,
                        scalar2=bias[:, mt:mt + 1],
                        op0=mybir.AluOpType.mult,
                        op1=mybir.AluOpType.add,
                    )

                def lastmm(mm=mm, drain=drain):
                    mm(2, 3)
                    push(CUR[0] + 2, [(0, drain)])

                units = [
                    (107, lambda t=t, j=j, mm=mm: mm(t, j))
                    for t in range(3) for j in range(4)
                ]
                if defer:
                    return units[:-1] + [(107, lastmm)]
                return units + [(0, drain)]

            # ---- prologue: warm-up matmuls keep the PE p-state ramping
            # through the DMA-bound head; then the pr0 nt0 K/Q chains ----
            wz = const.tile([NP, 512], F16, tag="wz")
            nc.gpsimd.memset(wz[:], 0.0)
            wps = psB.tile([NP, 512], F32, tag="proj", name="wps")
            for _ in range(N_WARM):
                nc.tensor.matmul(wps[:], wz[:, 0:NP], wz[:], start=True, stop=True)
            for _, u in qk_chain(0, 0, 0, defer=False) + qk_chain(0, 1, 0, defer=False):
                u()

            # ---- filler backlog, prioritized by need-slot ----
            backlog = []
            bseq = [0]

            def push(need, units):
                heapq.heappush(backlog, [need, bseq[0], units, [0]])
                bseq[0] += 1

            ventries = {}

            def pushv(need, p, st):
                entry = [need, bseq[0], v_chain(p, st), [0]]
                bseq[0] += 1
                ventries[(p, st)] = entry
                heapq.heappush(backlog, entry)

            # pr0: K nt1-3 land just after their x8/x8r chunks; Q nt per its
            # first consumer slot; V once wv/wvr are in (~slot 13)
            for nt, kn in ((1, 1), (2, 4), (3, 8)):
                push(kn, qk_chain(0, 0, nt))
            for nt, qn in ((1, 8), (2, 22), (3, 38)):
                push(qn, qk_chain(0, 1, nt))
            for st in range(NST):
                pushv(14 + st * 5 // 4, 0, st)
            for p in (1, 2, 3):
                push(64 * p - 22, qk_chain(p, 0, 0))
                push(64 * p - 14, qk_chain(p, 1, 0))
                for j in (1, 2, 3):
                    push(64 * p + 4 * j - 4, qk_chain(p, 0, j))
                for nt in (1, 2, 3):
                    push(64 * p + 16 * nt - 10, qk_chain(p, 1, nt))
                for st in range(NST):
                    pushv(64 * p - 24 + st, p, st)

            def pop_entry_units(entry, budget):
                need, seq, units, idx = entry
                spent = 0
                while idx[0] < len(units):
                    cost, u = units[idx[0]]
                    if spent > 0 and spent + cost > budget:
                        break
                    u()
                    spent += cost
                    idx[0] += 1
                return spent, idx[0] >= len(units)

            def force(need):
                while backlog and backlog[0][0] <= need:
                    entry = heapq.heappop(backlog)
                    pop_entry_units(entry, 10 ** 9)

            def drip(budget):
                while backlog and budget > 0:
                    # pop BEFORE running: units may push new entries (deferred
                    # drains) with lower need than this one
                    entry = heapq.heappop(backlog)
                    spent, done = pop_entry_units(entry, budget)
                    budget -= spent
                    if not done:
                        heapq.heappush(backlog, entry)
                        break

            # ---- ctx cursor ----
            pt_slots = {}
            ctx_c = [0]

            def emit_boundary(cq):
                # stage 1: drain the ctx PSUM chains (DVE); stage 2 (obs
                # normalize + stores) is deferred 2 more slots so the Pool
                # muls / store DMAs never head-block their queues waiting
                # on stage 1
                pr, qt = divmod(cq, 4)
                css, rcs = [], []
                for i in range(2):
                    cs = cspool.tile([NP, 260], F32, tag="cs", name="cs")
                    nc.vector.tensor_copy(cs[:], ctxps[i][:, 0:260])
                    rc = rcpool.tile([NP, 4], F32, tag="rc", name="rc")
                    nc.vector.reciprocal(
                        rc[:],
                        cs[:].rearrange("p (j e) -> p j e", e=HD + 1)[:, :, HD],
                    )
                    css.append(cs)
                    rcs.append(rc)
                push(CUR[0] + 2, [(0, lambda: emit_obs(cq, css, rcs))])

            def emit_obs(cq, css, rcs):
                pr, qt = divmod(cq, 4)
                oeng = nc.vector if cq >= 14 else nc.gpsimd
                for j in range(4):
                    for i in range(2):
                        hh = 2 * pr + i
                        oeng.tensor_scalar_mul(
                            obs[qt][:, j * GC + hh * HD:j * GC + (hh + 1) * HD],
                            css[i][:, 65 * j:65 * j + HD],
                            rcs[i][:, j:j + 1],
                        )
                    if pr == 3:
                        jq = qt * 4 + j
                        eng = nc.scalar if (qt == 3 and j % 2 == 1) else nc.sync
                        eng.dma_start(
                            out=out[jq * NP:(jq + 1) * NP, :],
                            in_=obs[qt][:, j * GC:(j + 1) * GC],
                        )

            bnd_emitted = {}

            def emit_ctx_group(c):
                cq, ck = divmod(c, NKT)
                pr = cq // 4
                pt = pt_slots.pop(c)
                for i in range(2):
                    for j in range(4):
                        nc.tensor.matmul(
                            ctxps[i][:, 65 * j:65 * j + 65],
                            pt[:, i * 512 + j * NP:i * 512 + (j + 1) * NP],
                            v_sb[pr][ck][:, i * 65:(i + 1) * 65],
                            start=(ck == 0 and j == 0),
                            stop=(ck == NKT - 1),
                            skip_group_check=True,
                        )
                if ck == NKT - 1:
                    # defer the boundary drain ~2 slots: its cs copies wait
                    # on this chain's last matmuls; emitting late keeps the
                    # DVE queue head unblocked
                    def bnd(cq=cq):
                        bnd_emitted[cq] = CUR[0]
                        emit_boundary(cq)

                    push(CUR[0] + 2, [(0, bnd)])

            def ctx_due(pos):
                c = ctx_c[0]
                if c >= min(pos, 256):
                    return False
                cq, ck = divmod(c, NKT)
                if ck == 0 and cq > 0:
                    # ctxps reuse: wait until the previous boundary's cs
                    # copies are emitted and have cleared the DVE queue
                    if cq - 1 not in bnd_emitted or pos < bnd_emitted[cq - 1] + BND_GATE:
                        return False
                return (cq // 4, ck) in v_done

            # ---- the global stream ----
            pos = 0
            for pr in range(4):
                for qt in range(NQ):
                    for kt in range(NKT):
                        CUR[0] = pos
                        force(pos)
                        pt = ptpool.tile([NP, 1024], F16, tag="pt", name="pt")
                        for i in range(2):
                            sc = psS.tile([NP, 512], F32, tag="sc", name="sc")
                            nc.tensor.matmul(
                                sc[:],
                                kt_sb[pr][i * 64:(i + 1) * 64,
                                          kt * NP:(kt + 1) * NP],
                                qt_sb[pr][i * 64:(i + 1) * 64,
                                          qt * 512:(qt + 1) * 512],
                                start=True,
                                stop=True,
                                tile_position=(i * 64, 0),
                            )
                            half = pt[:, i * 512:(i + 1) * 512]
                            if SCH_PAT[(2 * pos + i) % len(SCH_PAT)] and pos < 254:
                                ia = iapool.tile([NP, 512], I16, tag="ia",
                                                 name="ia")
                                nc.vector.tensor_scalar(
                                    out=ia[:], in0=sc[:], scalar1=A2S,
                                    scalar2=B2, op0=mybir.AluOpType.mult,
                                    op1=mybir.AluOpType.add,
                                )
                                ib = ibpool.tile([NP, 512], I16, tag="ib",
                                                 name="ib")
                                nc.vector.tensor_scalar(
                                    out=ib[:], in0=ia[:], scalar1=512,
                                    scalar2=None, op0=mybir.AluOpType.subtract,
                                )
                                nc.vector.tensor_tensor(
                                    out=half,
                                    in0=ia[:].bitcast(F16),
                                    in1=ib[:].bitcast(F16),
                                    op=mybir.AluOpType.mult,
                                )
                            else:
                                nc.scalar.activation(half, sc[:], EXPF,
                                                     scale=SCALE)
                        pt_slots[pos] = pt
                        pos += 1
                        n_ctx = 2 + (pos > 150) + (pos > 210)
                        emitted = 0
                        while emitted < n_ctx and ctx_due(pos):
                            emit_ctx_group(ctx_c[0])
                            ctx_c[0] += 1
                            emitted += 1
                        # fillers rationed so the backlog lasts the whole
                        # stream (total filler ~62us / 256 slots)
                        drip(DRIP_NS)
                        while pos - ctx_c[0] >= CTX_LAG:
                            cq, ck = divmod(ctx_c[0], NKT)
                            ve = ventries.get((cq // 4, ck))
                            if ve is not None:
                                force(ve[0])
                            if not ctx_due(pos):
                                force(pos + 2)  # pull deferred drains
                            if not ctx_due(pos):
                                break
                            emit_ctx_group(ctx_c[0])
                            ctx_c[0] += 1
            # tail: flush whatever is left
            CUR[0] = 260
            while ctx_c[0] < 256:
                force(10 ** 9)
                assert ctx_due(10 ** 9), f"ctx stuck at {ctx_c[0]}"
                emit_ctx_group(ctx_c[0])
                ctx_c[0] += 1
            force(10 ** 9)


_NC_CACHE = {}


def _get_nc():
    if "nc" not in _NC_CACHE:
        nc = bacc.Bacc("TRN2", target_bir_lowering=False, debug=False,
                       enable_asserts=False)
        x8 = nc.dram_tensor("x8", [NP, 16384], F8E4, kind="ExternalInput").ap()
        x8r = nc.dram_tensor("x8r", [NP, 16384], F8E5, kind="ExternalInput").ap()
        wq8 = nc.dram_tensor("wq8", [NP, 4096], F8E4, kind="ExternalInput").ap()
        wq8r = nc.dram_tensor("wq8r", [NP, 4096], F8E5, kind="ExternalInput").ap()
        wk8 = nc.dram_tensor("wk8", [NP, 4096], F8E4, kind="ExternalInput").ap()
        wk8r = nc.dram_tensor("wk8r", [NP, 4096], F8E5, kind="ExternalInput").ap()
        wv8 = nc.dram_tensor("wv8", [NP, 4096], F8E4, kind="ExternalInput").ap()
        wv8r = nc.dram_tensor("wv8r", [NP, 4096], F8E5, kind="ExternalInput").ap()
        bq = nc.dram_tensor("bq", [GC], F32, kind="ExternalInput").ap()
        bk = nc.dram_tensor("bk", [GC], F32, kind="ExternalInput").ap()
        bv = nc.dram_tensor("bv", [NP, GC], F32, kind="ExternalInput").ap()
        mask = nc.dram_tensor("mask", [S], F32, kind="ExternalInput").ap()
        out = nc.dram_tensor("out", [S, GC], F16, kind="ExternalOutput").ap()
        with tile.TileContext(nc) as tc:
            _emit(tc, x8, x8r, wq8, wq8r, wk8, wk8r, wv8, wv8r, bq, bk, bv,
                  mask, out)
        nc.compile()
        _NC_CACHE["nc"] = nc
    return _NC_CACHE["nc"]


def _pack_x(x16):
    """[2048 s, 1024 c] -> [128 p, (sblk j i s512)] fp8 pair (e4m3, e5m2)."""
    from ml_dtypes import float8_e4m3, float8_e5m2

    x8 = x16.astype(float8_e4m3)
    x8r = (x16.astype(np.float32) - x8.astype(np.float32)).astype(float8_e5m2)

    def lay(a):
        # c = 256j + 128i + p ; s = 512*sblk + s'
        t = np.ascontiguousarray(a.T)              # [1024 c, 2048 s]
        t = t.reshape(4, 2, 128, 4, 512)           # [j, i, p, sblk, s']
        t = t.transpose(2, 3, 0, 1, 4)             # [p, sblk, j, i, s']
        return np.ascontiguousarray(t.reshape(128, 16384))

    return lay(x8), lay(x8r)


def _pack_w(w16):
    """[1024 c, 512 m] -> [128 p, (mt j i m)] fp8 pair, prescaled x16."""
    from ml_dtypes import float8_e4m3, float8_e5m2

    ws = w16.astype(np.float32) * 16.0
    w8 = ws.astype(float8_e4m3)
    w8r = (ws - w8.astype(np.float32)).astype(float8_e5m2)

    def lay(a):
        t = a.reshape(4, 2, 128, 4, 128)           # [j, i, p, mt, m]
        t = t.transpose(2, 3, 0, 1, 4)             # [p, mt, j, i, m]
        return np.ascontiguousarray(t.reshape(128, 4096))

    return lay(w8), lay(w8r)


def _in_maps(inputs):
    hs = np.asarray(inputs["hidden_states"], np.float32)
    am = np.asarray(inputs["attention_mask"], np.float32)
    ws = {k: np.asarray(inputs[k], np.float32) for k in ("Wq", "Wk", "Wv")}
    bs = {k: np.asarray(inputs[k], np.float32) for k in ("bq", "bk", "bv")}
    maps = []
    for c in range(8):
        b, g = c // 2, c % 2
        cols = slice(g * GC, (g + 1) * GC)
        x8, x8r = _pack_x(hs[b].astype(np.float16))
        m = {"x8": x8, "x8r": x8r}
        for nm, wn in (("q", "Wq"), ("k", "Wk"), ("v", "Wv")):
            w8, w8r = _pack_w(ws[wn][:, cols].astype(np.float16))
            m[f"w{nm}8"] = w8
            m[f"w{nm}8r"] = w8r
        m["bq"] = np.ascontiguousarray(bs["bq"][cols])
        m["bk"] = np.ascontiguousarray(bs["bk"][cols])
        m["bv"] = np.ascontiguousarray(np.broadcast_to(bs["bv"][cols], (NP, GC)))
        m["mask"] = np.ascontiguousarray(am[b, 0, 0, :])
        maps.append(m)
    return maps


class _Runner:
    """Cached PJRT executor for the SPMD bass program (8 cores)."""

    def __init__(self, nc, n_cores=8):
        import jax
        from jax.experimental.shard_map import shard_map
        from jax.sharding import Mesh, PartitionSpec

        from concourse import bass2jax, mybir as _mybir

        bass2jax.install_neuronx_cc_hook()
        self.jax = jax
        self.nc = nc
        self.n_cores = n_cores
        assert nc.dbg_addr is None
        part_name = (
            nc.partition_id_tensor.name if nc.partition_id_tensor is not None else None
        )

        in_names, out_names, out_avals, zero_outs = [], [], [], []
        for alloc in nc.m.functions[0].allocations:
            if not isinstance(alloc, _mybir.MemoryLocationSet):
                continue
            name = alloc.memorylocations[0].name
            if alloc.kind == "ExternalInput":
                if name != part_name:
                    in_names.append(name)
            elif alloc.kind == "ExternalOutput":
                out_names.append(name)
                shape = tuple(alloc.tensor_shape)
                dtype = _mybir.dt.np(alloc.dtype)
                out_avals.append(jax.core.ShapedArray(shape, dtype))
                zero_outs.append(np.zeros(shape, dtype))
        self.in_names = list(in_names)
        self.out_names = list(out_names)
        self.out_avals = out_avals
        self.zero_outs = zero_outs
        n_params, n_outs = len(in_names), len(out_avals)
        all_names = in_names + out_names
        if part_name is not None:
            all_names = all_names + [part_name]
        donate = tuple(range(n_params, n_params + n_outs))

        def _body(*args):
            operands = list(args)
            if part_name is not None:
                operands.append(bass2jax.partition_id_tensor())
            outs = bass2jax._bass_exec_p.bind(
                *operands,
                out_avals=tuple(out_avals),
                in_names=tuple(all_names),
                out_names=tuple(out_names),
                lowering_input_output_aliases=(),
                sim_require_finite=True,
                sim_require_nnan=True,
                nc=nc,
            )
            return tuple(outs)

        self._body = _body
        devices = jax.devices()[:n_cores]
        self.mesh = Mesh(np.asarray(devices), ("core",))
        self.pspec = PartitionSpec("core")
        in_specs = (self.pspec,) * (n_params + n_outs)
        out_specs = (self.pspec,) * n_outs
        self.sharded = jax.jit(
            shard_map(
                _body,
                mesh=self.mesh,
                in_specs=in_specs,
                out_specs=out_specs,
                check_rep=False,
            ),
            donate_argnums=donate,
            keep_unused=True,
        )

    def concat_inputs(self, in_maps):
        return [
            np.concatenate([np.asarray(m[name]) for m in in_maps], axis=0)
            for name in self.in_names
        ]

    def fresh_zeros(self):
        return [
            np.zeros((self.n_cores * z.shape[0], *z.shape[1:]), z.dtype)
            for z in self.zero_outs
        ]

    def __call__(self, in_maps):
        out_arrs = self.sharded(*self.concat_inputs(in_maps), *self.fresh_zeros())
        return [
            {
                name: np.asarray(out_arrs[i]).reshape(
                    self.n_cores, *self.out_avals[i].shape
                )[c]
                for i, name in enumerate(self.out_names)
            }
            for c in range(self.n_cores)
        ]


def _get_runner():
    if "runner" not in _NC_CACHE:
        _NC_CACHE["runner"] = _Runner(_get_nc())
    return _NC_CACHE["runner"]


def _assemble(results):
    full = np.empty((B, S, H), np.float32)
    for c in range(8):
        b, g = c // 2, c % 2
        full[b, :, g * GC:(g + 1) * GC] = results[c]["out"].astype(np.float32)
    return full


def _run(inputs, trace=False, **kwargs):
    if trace:
        from concourse.bass_utils import run_bass_kernel_spmd

        nc = _get_nc()
        res = run_bass_kernel_spmd(
            nc, _in_maps(inputs), core_ids=list(range(8)), trace=True, **kwargs
        )
        return _assemble(res.results), res

    return _assemble(_get_runner()(_in_maps(inputs))), None


def kernel(**inputs):
    return _run(inputs)[0]


# revision 62
# speedup vs baseline: 1.0010x; 1.0010x over previous
"""BertSelfAttention Trainium2 Bass kernel (v2).

Full inputs in, full output out. Sharding: 8 cores = 4 batches x 2 head
groups (8 heads each). Per-core SPMD program (no collectives).

v2 structure (vs the v1 fp16 baseline at 301us):
  - Projections run in fp8 DoubleRow (cost-model 0.5 cyc/row, contraction
    256/instr): Q = (x8 + x8r) @ (w8 + w8r) dropping the rr term. Main
    tensors e4m3 (W host-prescaled x16 out of the subnormal range),
    residuals e5m2 (wide exponent range); all three terms share one PSUM
    scale so a single drain rescales by 1/16. X^T is host-transposed into
    the DoubleRow layout, removing all on-device transposes.
  - Scores and ctx stay fp16 (fp8 noise there lands ~3-6% in the final
    metric - over the 2e-2 gate). PE ~226us is the design bottleneck.
  - exp is split: each slot's two score matmuls write separate [128,512]
    PSUM tiles (ring of 4); per half-tile a fixed 32-period pattern
    assigns ACT (exact exp, 75%) or a DVE 2-term Schraudolph (25%):
    ia = trunc(score*A + B) int16 straight from PSUM, ib = ia - 512,
    pt = f16bits(ia) * f16bits(ib); the pair product cancels most of
    the linear-interp error (rms ~0.5%). Strict A/S interleaving keeps
    the exp engines off each other's ring slots. (GPSIMD cannot touch
    PSUM on real hw, so Pool only gets SBUF-side work: obs normalize,
    em scaling, memsets.)
  - ctx accumulated as in v1: [128 q, 65] chains packed into 2 PSUM
    banks, trailing cursor on a 16-deep pt ring; projection/boundary
    drains are emitted ~2 slots late so they never head-block the
    in-order DVE queue on a still-running PE chain.
"""

import sys
from contextlib import ExitStack

import numpy as np

sys.path.insert(0, "/opt/trn_rl_repo")

import concourse.bass as bass  # noqa: E402
from concourse import bacc  # noqa: E402
import concourse.mybir as mybir  # noqa: E402
import concourse.tile as tile  # noqa: E402

B, S, H = 4, 2048, 1024
NH, HD = 16, 64
GH = 8            # heads per core
GC = GH * HD      # 512 output cols per core
NP = 128          # partitions
NQ = S // 512     # 4 q blocks of 512
NKT = S // NP     # 16 k tiles of 128
NST = 16
F32 = mybir.dt.float32
F16 = mybir.dt.float16
I16 = mybir.dt.int16
F8E4 = mybir.dt.float8e4
F8E5 = mybir.dt.float8e5
SCALE = 1.0 / 8.0  # 1/sqrt(HD)
EXPF = mybir.ActivationFunctionType.Exp
DR = mybir.MatmulPerfMode.DoubleRow

# 2-term Schraudolph constants: ia = trunc(score * A2S + B2) (int16),
# exp(score/8) ~= f16bits(ia) * f16bits(ia - 512).
import math as _math
A2S = float(SCALE * 512.0 / _math.log(2.0))
B2 = 15561.25
WSCL = 1.0 / 16.0  # W host-prescale compensation in drains

import os as _os
N_WARM = int(_os.environ.get("K_NWARM", 11))
PT_RING = int(_os.environ.get("K_PTRING", 16))
CTX_LAG = int(_os.environ.get("K_CTXLAG", 14))
DRIP_NS = int(_os.environ.get("K_DRIP", 250))
BND_GATE = int(_os.environ.get("K_BGATE", 3))
# per-half exp engine pattern, period 32 over half-index (2*pos + i):
# True = DVE 2-term Schraudolph (8/32), False = exact ACT exp. Stride-4
# placement on odd halves keeps ACT/DVE strictly interleaved on the
# 4-deep PSUM score ring.
_S32S = {
    "8": {1, 5, 9, 13, 17, 21, 25, 29},
    "9o": {1, 3, 5, 9, 13, 17, 21, 25, 29},
    "9b": {1, 5, 9, 11, 15, 19, 23, 27, 31},
    "8p3": {3, 7, 11, 15, 19, 23, 27, 31},
    "10o": {1, 3, 7, 11, 13, 17, 21, 23, 27, 31},
    "9c": {1, 5, 7, 11, 15, 19, 23, 27, 31},
    "9d": {1, 5, 9, 13, 15, 19, 23, 27, 31},
    "9e": {3, 5, 9, 13, 17, 19, 23, 27, 31},
    "9f": {1, 5, 9, 13, 15, 19, 23, 27, 29},
    "9g": {1, 3, 7, 11, 15, 19, 23, 27, 31},
    "9h": {1, 5, 9, 13, 15, 17, 21, 25, 29},
}
_S32 = _S32S[_os.environ.get("K_S32", "9d")]
SCH_PAT = tuple(i in _S32 for i in range(32))


def _emit(tc, x8, x8r, wq8, wq8r, wk8, wk8r, wv8, wv8r, bqd, bkd, bvd,
          maskd, out):
    import heapq

    nc = tc.nc
    with ExitStack() as ctx:
        const = ctx.enter_context(tc.tile_pool(name="const", bufs=1))
        big = ctx.enter_context(tc.tile_pool(name="big", bufs=1))

        # ---- consts ----
        mask_sb = const.tile([NP, NKT], F32, tag="mask")
        em = const.tile([NP, NKT], F32, tag="em")
        bk_sb = const.tile([NP, 4], F32, tag="bk")
        bq_sb = const.tile([NP, 4], F32, tag="bq")
        bv_bc = const.tile([NP, GC], F32, tag="bvbc")

        # persistent SBUF tensors
        x8_sb = big.tile([NP, 16384], F8E4, tag="x8", name="x8")
        x8r_sb = big.tile([NP, 16384], F8E5, tag="x8r", name="x8r")
        qt_sb = [big.tile([NP, S], F16, tag=f"qt{i}", name=f"qt{i}") for i in range(4)]
        kt_sb = [big.tile([NP, S], F16, tag=f"kt{i}", name=f"kt{i}") for i in range(4)]
        v_sb = [
            [
                big.tile([NP, 2 * (HD + 1)], F16, tag=f"v{p}_{i}", name=f"v{p}_{i}")
                for i in range(NST)
            ]
            for p in range(4)
        ]
        wq_sb = big.tile([NP, 4096], F8E4, tag="wq")
        wqr_sb = big.tile([NP, 4096], F8E5, tag="wqr")
        wk_sb = big.tile([NP, 4096], F8E4, tag="wk")
        wkr_sb = big.tile([NP, 4096], F8E5, tag="wkr")
        wv_sb = big.tile([NP, 4096], F8E4, tag="wv")
        wvr_sb = big.tile([NP, 4096], F8E5, tag="wvr")

        def ld(dst, src, c0, cw):
            nc.sync.dma_start(out=dst[:, c0:c0 + cw], in_=src[:, c0:c0 + cw])

        # ---- DMA queue, ordered by first use; each DMACopy holds SP.SEQ
        # ~650ns, so the fat pr0-chain inputs issue FIRST and the tiny
        # bias/mask transfers (needed only at drain time ~5.5us) follow ----
        ld(wk_sb, wk8, 0, 1024)    # mt0 slices first: pr0 K/Q chains
        ld(wq_sb, wq8, 0, 1024)
        ld(x8_sb, x8, 0, 4096)     # s-block 0
        ld(wkr_sb, wk8r, 0, 1024)
        ld(wqr_sb, wq8r, 0, 1024)
        ld(x8r_sb, x8r, 0, 4096)
        nc.sync.dma_start(out=bk_sb[:], in_=bkd.rearrange("(m p) -> p m", p=NP))
        nc.sync.dma_start(out=bq_sb[:], in_=bqd.rearrange("(m p) -> p m", p=NP))
        nc.sync.dma_start(out=mask_sb[:], in_=maskd.rearrange("(t p) -> p t", p=NP))
        nc.scalar.activation(em[:], mask_sb[:], EXPF)  # warms Exp table too
        for sb in range(1, 4):     # s-blocks 1-3 (scores kt=4sb need them)
            ld(x8_sb, x8, sb * 4096, 4096)
            ld(x8r_sb, x8r, sb * 4096, 4096)
        ld(wv_sb, wv8, 0, 4096)
        ld(wvr_sb, wv8r, 0, 4096)
        nc.sync.dma_start(out=bv_bc[:], in_=bvd)
        ld(wk_sb, wk8, 1024, 3072)
        ld(wq_sb, wq8, 1024, 3072)
        ld(wkr_sb, wk8r, 1024, 3072)
        ld(wqr_sb, wq8r, 1024, 3072)

        def w_ap(w, mt, j):
            base = (mt * 4 + j) * 256
            return w[:, base:base + 256].rearrange("p (i m) -> p i m", i=2)

        def x_ap(xsb, sblk, j):
            base = (sblk * 4 + j) * 1024
            return xsb[:, base:base + 1024].rearrange("p (i s) -> p i s", i=2)

        with (
            tc.tile_pool(name="psS", bufs=4, space="PSUM") as psS,
            tc.tile_pool(name="psC", bufs=1, space="PSUM") as psC,
            tc.tile_pool(name="psB", bufs=2, space="PSUM") as psB,
            tc.tile_pool(name="ptpool", bufs=PT_RING) as ptpool,
            tc.tile_pool(name="iapool", bufs=3) as iapool,
            tc.tile_pool(name="ibpool", bufs=3) as ibpool,
            tc.tile_pool(name="cspool", bufs=4) as cspool,
            tc.tile_pool(name="rcpool", bufs=4) as rcpool,
            tc.tile_pool(name="obpool", bufs=1) as obpool,
        ):
            ctxps = [
                psC.tile([NP, 512], F32, tag=f"ctx{i}", name=f"ctx{i}")
                for i in range(2)
            ]
            obs = {
                qt: obpool.tile([NP, 4 * GC], F16, tag=f"ob{qt}", name="ob")
                for qt in range(NQ)
            }

            v_done = set()
            CUR = [0]       # current stream slot (for deferred drains)

            # terms: (w8,x8), (w8r,x8), (w8,x8r) - x8r arrives last via DMA
            _terms = ((0, 0), (1, 0), (0, 1))

            def v_chain(pr, st):
                """Project V cols for head pair pr, s-tile st (fp8 3-term)."""
                cell = {}
                sblk, sw = divmod(st, 4)

                def mm(t, j, cell=cell):
                    if t == 0 and j == 0:
                        cell["pv"] = psB.tile(
                            [NP, 512], F32, tag="proj", name="pv"
                        )[:, 0:NP]
                    wt, xt_ = _terms[t]
                    w = (wv_sb, wvr_sb)[wt]
                    xx = (x8_sb, x8r_sb)[xt_]
                    nc.tensor.matmul(
                        cell["pv"],
                        x_ap(xx, sblk, j)[:, :, sw * NP:(sw + 1) * NP],
                        w_ap(w, pr, j),
                        start=(t == 0 and j == 0),
                        stop=(t == 2 and j == 3),
                        perf_mode=DR,
                    )

                def drain(cell=cell):
                    v3 = v_sb[pr][st][:].rearrange("p (h e) -> p h e", e=HD + 1)
                    nc.gpsimd.memset(v3[:, :, HD], 1.0)
                    nc.vector.scalar_tensor_tensor(
                        out=v3[:, :, 0:HD],
                        in0=cell["pv"].rearrange("p (h e) -> p h e", e=HD),
                        scalar=WSCL,
                        in1=bv_bc[:, pr * NP:(pr + 1) * NP].rearrange(
                            "p (h e) -> p h e", e=HD
                        ),
                        op0=mybir.AluOpType.mult,
                        op1=mybir.AluOpType.add,
                    )
                    nc.gpsimd.tensor_scalar_mul(
                        v_sb[pr][st][:], v_sb[pr][st][:], em[:, st:st + 1]
                    )
                    v_done.add((pr, st))

                def lastmm(mm=mm, drain=drain):
                    # defer the drain ~2 slots so it never head-blocks the
                    # DVE queue waiting on this chain's PE completion
                    mm(2, 3)
                    push(CUR[0] + 2, [(0, drain)])

                return [
                    (27, lambda t=t, j=j, mm=mm: mm(t, j))
                    for t in range(3) for j in range(4)
                ][:-1] + [(27, lastmm)]

            def qk_chain(mt, which, nt, defer=True):
                wm, wr, dst, bias = (
                    (wk_sb, wkr_sb, kt_sb, bk_sb),
                    (wq_sb, wqr_sb, qt_sb, bq_sb),
                )[which]
                cell = {}

                def mm(t, j, cell=cell):
                    if t == 0 and j == 0:
                        cell["pp"] = psB.tile([NP, 512], F32, tag="proj", name="pp")
                    wt, xt_ = _terms[t]
                    w = (wm, wr)[wt]
                    xx = (x8_sb, x8r_sb)[xt_]
                    nc.tensor.matmul(
                        cell["pp"][:],
                        w_ap(w, mt, j),
                        x_ap(xx, nt, j),
                        start=(t == 0 and j == 0),
                        stop=(t == 2 and j == 3),
                        perf_mode=DR,
                    )

                def drain(cell=cell):
                    nc.vector.tensor_scalar(
                        out=dst[mt][:, nt * 512:(nt + 1) * 512],
                        in0=cell["pp"][:],
                        scalar1=WSCL,
                        scalar2=bias[:, mt:mt + 1],
                        op0=mybir.AluOpType.mult,
                        op1=mybir.AluOpType.add,
                    )

                def lastmm(mm=mm, drain=drain):
                    mm(2, 3)
                    push(CUR[0] + 2, [(0, drain)])

                units = [
                    (107, lambda t=t, j=j, mm=mm: mm(t, j))
                    for t in range(3) for j in range(4)
                ]
                if defer:
                    return units[:-1] + [(107, lastmm)]
                return units + [(0, drain)]

            # ---- prologue: warm-up matmuls keep the PE p-state ramping
            # through the DMA-bound head; then the pr0 nt0 K/Q chains ----
            wz = const.tile([NP, 512], F16, tag="wz")
            nc.gpsimd.memset(wz[:], 0.0)
            wps = psB.tile([NP, 512], F32, tag="proj", name="wps")
            for _ in range(N_WARM):
                nc.tensor.matmul(wps[:], wz[:, 0:NP], wz[:], start=True, stop=True)
            for _, u in qk_chain(0, 0, 0, defer=False) + qk_chain(0, 1, 0, defer=False):
                u()

            # ---- filler backlog, prioritized by need-slot ----
            backlog = []
            bseq = [0]

            def push(need, units):
                heapq.heappush(backlog, [need, bseq[0], units, [0]])
                bseq[0] += 1

            ventries = {}

            def pushv(need, p, st):
                entry = [need, bseq[0], v_chain(p, st), [0]]
                bseq[0] += 1
                ventries[(p, st)] = entry
                heapq.heappush(backlog, entry)

            # pr0: K nt1-3 land just after their x8/x8r chunks; Q nt per its
            # first consumer slot; V once wv/wvr are in (~slot 13)
            for nt, kn in ((1, 1), (2, 4), (3, 8)):
                push(kn, qk_chain(0, 0, nt))
            for nt, qn in ((1, 8), (2, 22), (3, 38)):
                push(qn, qk_chain(0, 1, nt))
            for st in range(NST):
                pushv(14 + st * 5 // 4, 0, st)
            for p in (1, 2, 3):
                push(64 * p - 22, qk_chain(p, 0, 0))
                push(64 * p - 14, qk_chain(p, 1, 0))
                for j in (1, 2, 3):
                    push(64 * p + 4 * j - 4, qk_chain(p, 0, j))
                for nt in (1, 2, 3):
                    push(64 * p + 16 * nt - 10, qk_chain(p, 1, nt))
                for st in range(NST):
                    pushv(64 * p - 24 + st, p, st)

            def pop_entry_units(entry, budget):
                need, seq, units, idx = entry
                spent = 0
                while idx[0] < len(units):
                    cost, u = units[idx[0]]
                    if spent > 0 and spent + cost > budget:
                        break
                    u()
                    spent += cost
                    idx[0] += 1
                return spent, idx[0] >= len(units)

            def force(need):
                while backlog and backlog[0][0] <= need:
                    entry = heapq.heappop(backlog)
                    pop_entry_units(entry, 10 ** 9)

            def drip(budget):
                while backlog and budget > 0:
                    # pop BEFORE running: units may push new entries (deferred
                    # drains) with lower need than this one
                    entry = heapq.heappop(backlog)
                    spent, done = pop_entry_units(entry, budget)
                    budget -= spent
                    if not done:
                        heapq.heappush(backlog, entry)
                        break

            # ---- ctx cursor ----
            pt_slots = {}
            ctx_c = [0]

            def emit_boundary(cq):
                # stage 1: drain the ctx PSUM chains (DVE); stage 2 (obs
                # normalize + stores) is deferred 2 more slots so the Pool
                # muls / store DMAs never head-block their queues waiting
                # on stage 1
                pr, qt = divmod(cq, 4)
                css, rcs = [], []
                for i in range(2):
                    cs = cspool.tile([NP, 260], F32, tag="cs", name="cs")
                    nc.vector.tensor_copy(cs[:], ctxps[i][:, 0:260])
                    rc = rcpool.tile([NP, 4], F32, tag="rc", name="rc")
                    nc.vector.reciprocal(
                        rc[:],
                        cs[:].rearrange("p (j e) -> p j e", e=HD + 1)[:, :, HD],
                    )
                    css.append(cs)
                    rcs.append(rc)
                push(CUR[0] + 2, [(0, lambda: emit_obs(cq, css, rcs))])

            def emit_obs(cq, css, rcs):
                pr, qt = divmod(cq, 4)
                oeng = nc.vector if cq >= 14 else nc.gpsimd
                for j in range(4):
                    for i in range(2):
                        hh = 2 * pr + i
                        oeng.tensor_scalar_mul(
                            obs[qt][:, j * GC + hh * HD:j * GC + (hh + 1) * HD],
                            css[i][:, 65 * j:65 * j + HD],
                            rcs[i][:, j:j + 1],
                        )
                    if pr == 3:
                        jq = qt * 4 + j
                        eng = nc.scalar if (qt == 3 and j % 2 == 1) else nc.sync
                        eng.dma_start(
                            out=out[jq * NP:(jq + 1) * NP, :],
                            in_=obs[qt][:, j * GC:(j + 1) * GC],
                        )

            bnd_emitted = {}

            def emit_ctx_group(c):
                cq, ck = divmod(c, NKT)
                pr = cq // 4
                pt = pt_slots.pop(c)
                for i in range(2):
                    for j in range(4):
                        nc.tensor.matmul(
                            ctxps[i][:, 65 * j:65 * j + 65],
                            pt[:, i * 512 + j * NP:i * 512 + (j + 1) * NP],
                            v_sb[pr][ck][:, i * 65:(i + 1) * 65],
                            start=(ck == 0 and j == 0),
                            stop=(ck == NKT - 1),
                            skip_group_check=True,
                        )
                if ck == NKT - 1:
                    # defer the boundary drain ~2 slots: its cs copies wait
                    # on this chain's last matmuls; emitting late keeps the
                    # DVE queue head unblocked
                    def bnd(cq=cq):
                        bnd_emitted[cq] = CUR[0]
                        emit_boundary(cq)

                    push(CUR[0] + 2, [(0, bnd)])

            def ctx_due(pos):
                c = ctx_c[0]
                if c >= min(pos, 256):
                    return False
                cq, ck = divmod(c, NKT)
                if ck == 0 and cq > 0:
                    # ctxps reuse: wait until the previous boundary's cs
                    # copies are emitted and have cleared the DVE queue
                    if cq - 1 not in bnd_emitted or pos < bnd_emitted[cq - 1] + BND_GATE:
                        return False
                return (cq // 4, ck) in v_done

            # ---- the global stream ----
            pos = 0
            for pr in range(4):
                for qt in range(NQ):
                    for kt in range(NKT):
                        CUR[0] = pos
                        force(pos)
                        pt = ptpool.tile([NP, 1024], F16, tag="pt", name="pt")
                        for i in range(2):
                            sc = psS.tile([NP, 512], F32, tag="sc", name="sc")
                            nc.tensor.matmul(
                                sc[:],
                                kt_sb[pr][i * 64:(i + 1) * 64,
                                          kt * NP:(kt + 1) * NP],
                                qt_sb[pr][i * 64:(i + 1) * 64,
                                          qt * 512:(qt + 1) * 512],
                                start=True,
                                stop=True,
                                tile_position=(i * 64, 0),
                            )
                            half = pt[:, i * 512:(i + 1) * 512]
                            if SCH_PAT[(2 * pos + i) % len(SCH_PAT)]:
                                ia = iapool.tile([NP, 512], I16, tag="ia",
                                                 name="ia")
                                nc.vector.tensor_scalar(
                                    out=ia[:], in0=sc[:], scalar1=A2S,
                                    scalar2=B2, op0=mybir.AluOpType.mult,
                                    op1=mybir.AluOpType.add,
                                )
                                ib = ibpool.tile([NP, 512], I16, tag="ib",
                                                 name="ib")
                                nc.vector.tensor_scalar(
                                    out=ib[:], in0=ia[:], scalar1=512,
                                    scalar2=None, op0=mybir.AluOpType.subtract,
                                )
                                nc.vector.tensor_tensor(
                                    out=half,
                                    in0=ia[:].bitcast(F16),
                                    in1=ib[:].bitcast(F16),
                                    op=mybir.AluOpType.mult,
                                )
                            else:
                                nc.scalar.activation(half, sc[:], EXPF,
                                                     scale=SCALE)
                        pt_slots[pos] = pt
                        pos += 1
                        n_ctx = 2 + (pos > 150) + (pos > 210)
                        emitted = 0
                        while emitted < n_ctx and ctx_due(pos):
                            emit_ctx_group(ctx_c[0])
                            ctx_c[0] += 1
                            emitted += 1
                        # fillers rationed so the backlog lasts the whole
                        # stream (total filler ~62us / 256 slots)
                        drip(DRIP_NS)
                        while pos - ctx_c[0] >= CTX_LAG:
                            cq, ck = divmod(ctx_c[0], NKT)
                            ve = ventries.get((cq // 4, ck))
                            if ve is not None:
                                force(ve[0])
                            if not ctx_due(pos):
                                force(pos + 2)  # pull deferred drains
                            if not ctx_due(pos):
                                break
                            emit_ctx_group(ctx_c[0])
                            ctx_c[0] += 1
            # tail: flush whatever is left
            CUR[0] = 260
            while ctx_c[0] < 256:
                force(10 ** 9)
                assert ctx_due(10 ** 9), f"ctx stuck at {ctx_c[0]}"
                emit_ctx_group(ctx_c[0])
                ctx_c[0] += 1
            force(10 ** 9)


_NC_CACHE = {}


def _get_nc():
    if "nc" not in _NC_CACHE:
        nc = bacc.Bacc("TRN2", target_bir_lowering=False, debug=False,
                       enable_asserts=False)
        x8 = nc.dram_tensor("x8", [NP, 16384], F8E4, kind="ExternalInput").ap()
        x8r = nc.dram_tensor("x8r", [NP, 16384], F8E5, kind="ExternalInput").ap()
        wq8 = nc.dram_tensor("wq8", [NP, 4096], F8E4, kind="ExternalInput").ap()
        wq8r = nc.dram_tensor("wq8r", [NP, 4096], F8E5, kind="ExternalInput").ap()
        wk8 = nc.dram_tensor("wk8", [NP, 4096], F8E4, kind="ExternalInput").ap()
        wk8r = nc.dram_tensor("wk8r", [NP, 4096], F8E5, kind="ExternalInput").ap()
        wv8 = nc.dram_tensor("wv8", [NP, 4096], F8E4, kind="ExternalInput").ap()
        wv8r = nc.dram_tensor("wv8r", [NP, 4096], F8E5, kind="ExternalInput").ap()
        bq = nc.dram_tensor("bq", [GC], F32, kind="ExternalInput").ap()
        bk = nc.dram_tensor("bk", [GC], F32, kind="ExternalInput").ap()
        bv = nc.dram_tensor("bv", [NP, GC], F32, kind="ExternalInput").ap()
        mask = nc.dram_tensor("mask", [S], F32, kind="ExternalInput").ap()
        out = nc.dram_tensor("out", [S, GC], F16, kind="ExternalOutput").ap()
        with tile.TileContext(nc) as tc:
            _emit(tc, x8, x8r, wq8, wq8r, wk8, wk8r, wv8, wv8r, bq, bk, bv,
                  mask, out)
        nc.compile()
        _NC_CACHE["nc"] = nc
    return _NC_CACHE["nc"]


def _pack_x(x16):
    """[2048 s, 1024 c] -> [128 p, (sblk j i s512)] fp8 pair (e4m3, e5m2)."""
    from ml_dtypes import float8_e4m3, float8_e5m2

    x8 = x16.astype(float8_e4m3)
    x8r = (x16.astype(np.float32) - x8.astype(np.float32)).astype(float8_e5m2)

    def lay(a):
        # c = 256j + 128i + p ; s = 512*sblk + s'
        t = np.ascontiguousarray(a.T)              # [1024 c, 2048 s]
        t = t.reshape(4, 2, 128, 4, 512)           # [j, i, p, sblk, s']
        t = t.transpose(2, 3, 0, 1, 4)             # [p, sblk, j, i, s']
        return np.ascontiguousarray(t.reshape(128, 16384))

    return lay(x8), lay(x8r)


def _pack_w(w16):
    """[1024 c, 512 m] -> [128 p, (mt j i m)] fp8 pair, prescaled x16."""
    from ml_dtypes import float8_e4m3, float8_e5m2

    ws = w16.astype(np.float32) * 16.0
    w8 = ws.astype(float8_e4m3)
    w8r = (ws - w8.astype(np.float32)).astype(float8_e5m2)

    def lay(a):
        t = a.reshape(4, 2, 128, 4, 128)           # [j, i, p, mt, m]
        t = t.transpose(2, 3, 0, 1, 4)             # [p, mt, j, i, m]
        return np.ascontiguousarray(t.reshape(128, 4096))

    return lay(w8), lay(w8r)


def _in_maps(inputs):
    hs = np.asarray(inputs["hidden_states"], np.float32)
    am = np.asarray(inputs["attention_mask"], np.float32)
    ws = {k: np.asarray(inputs[k], np.float32) for k in ("Wq", "Wk", "Wv")}
    bs = {k: np.asarray(inputs[k], np.float32) for k in ("bq", "bk", "bv")}
    maps = []
    for c in range(8):
        b, g = c // 2, c % 2
        cols = slice(g * GC, (g + 1) * GC)
        x8, x8r = _pack_x(hs[b].astype(np.float16))
        m = {"x8": x8, "x8r": x8r}
        for nm, wn in (("q", "Wq"), ("k", "Wk"), ("v", "Wv")):
            w8, w8r = _pack_w(ws[wn][:, cols].astype(np.float16))
            m[f"w{nm}8"] = w8
            m[f"w{nm}8r"] = w8r
        m["bq"] = np.ascontiguousarray(bs["bq"][cols])
        m["bk"] = np.ascontiguousarray(bs["bk"][cols])
        m["bv"] = np.ascontiguousarray(np.broadcast_to(bs["bv"][cols], (NP, GC)))
        m["mask"] = np.ascontiguousarray(am[b, 0, 0, :])
        maps.append(m)
    return maps


class _Runner:
    """Cached PJRT executor for the SPMD bass program (8 cores)."""

    def __init__(self, nc, n_cores=8):
        import jax
        from jax.experimental.shard_map import shard_map
        from jax.sharding import Mesh, PartitionSpec

        from concourse import bass2jax, mybir as _mybir

        bass2jax.install_neuronx_cc_hook()
        self.jax = jax
        self.nc = nc
        self.n_cores = n_cores
        assert nc.dbg_addr is None
        part_name = (
            nc.partition_id_tensor.name if nc.partition_id_tensor is not None else None
        )

        in_names, out_names, out_avals, zero_outs = [], [], [], []
        for alloc in nc.m.functions[0].allocations:
            if not isinstance(alloc, _mybir.MemoryLocationSet):
                continue
            name = alloc.memorylocations[0].name
            if alloc.kind == "ExternalInput":
                if name != part_name:
                    in_names.append(name)
            elif alloc.kind == "ExternalOutput":
                out_names.append(name)
                shape = tuple(alloc.tensor_shape)
                dtype = _mybir.dt.np(alloc.dtype)
                out_avals.append(jax.core.ShapedArray(shape, dtype))
                zero_outs.append(np.zeros(shape, dtype))
        self.in_names = list(in_names)
        self.out_names = list(out_names)
        self.out_avals = out_avals
        self.zero_outs = zero_outs
        n_params, n_outs = len(in_names), len(out_avals)
        all_names = in_names + out_names
        if part_name is not None:
            all_names = all_names + [part_name]
        donate = tuple(range(n_params, n_params + n_outs))

        def _body(*args):
            operands = list(args)
            if part_name is not None:
                operands.append(bass2jax.partition_id_tensor())
            outs = bass2jax._bass_exec_p.bind(
                *operands,
                out_avals=tuple(out_avals),
                in_names=tuple(all_names),
                out_names=tuple(out_names),
                lowering_input_output_aliases=(),
                sim_require_finite=True,
                sim_require_nnan=True,
                nc=nc,
            )
            return tuple(outs)

        self._body = _body
        devices = jax.devices()[:n_cores]
        self.mesh = Mesh(np.asarray(devices), ("core",))
        self.pspec = PartitionSpec("core")
        in_specs = (self.pspec,) * (n_params + n_outs)
        out_specs = (self.pspec,) * n_outs
        self.sharded = jax.jit(
            shard_map(
                _body,
                mesh=self.mesh,
                in_specs=in_specs,
                out_specs=out_specs,
                check_rep=False,
            ),
            donate_argnums=donate,
            keep_unused=True,
        )

    def concat_inputs(self, in_maps):
        return [
            np.concatenate([np.asarray(m[name]) for m in in_maps], axis=0)
            for name in self.in_names
        ]

    def fresh_zeros(self):
        return [
            np.zeros((self.n_cores * z.shape[0], *z.shape[1:]), z.dtype)
            for z in self.zero_outs
        ]

    def __call__(self, in_maps):
        out_arrs = self.sharded(*self.concat_inputs(in_maps), *self.fresh_zeros())
        return [
            {
                name: np.asarray(out_arrs[i]).reshape(
                    self.n_cores, *self.out_avals[i].shape
                )[c]
                for i, name in enumerate(self.out_names)
            }
            for c in range(self.n_cores)
        ]


def _get_runner():
    if "runner" not in _NC_CACHE:
        _NC_CACHE["runner"] = _Runner(_get_nc())
    return _NC_CACHE["runner"]


def _assemble(results):
    full = np.empty((B, S, H), np.float32)
    for c in range(8):
        b, g = c // 2, c % 2
        full[b, :, g * GC:(g + 1) * GC] = results[c]["out"].astype(np.float32)
    return full


def _run(inputs, trace=False, **kwargs):
    if trace:
        from concourse.bass_utils import run_bass_kernel_spmd

        nc = _get_nc()
        res = run_bass_kernel_spmd(
            nc, _in_maps(inputs), core_ids=list(range(8)), trace=True, **kwargs
        )
        return _assemble(res.results), res

    return _assemble(_get_runner()(_in_maps(inputs))), None


def kernel(**inputs):
    return _run(inputs)[0]


# revision 64
# speedup vs baseline: 1.0035x; 1.0025x over previous
"""BertSelfAttention Trainium2 Bass kernel (v2).

Full inputs in, full output out. Sharding: 8 cores = 4 batches x 2 head
groups (8 heads each). Per-core SPMD program (no collectives).

v2 structure (vs the v1 fp16 baseline at 301us):
  - Projections run in fp8 DoubleRow (cost-model 0.5 cyc/row, contraction
    256/instr): Q = (x8 + x8r) @ (w8 + w8r) dropping the rr term. Main
    tensors e4m3 (W host-prescaled x16 out of the subnormal range),
    residuals e5m2 (wide exponent range); all three terms share one PSUM
    scale so a single drain rescales by 1/16. X^T is host-transposed into
    the DoubleRow layout, removing all on-device transposes.
  - Scores and ctx stay fp16 (fp8 noise there lands ~3-6% in the final
    metric - over the 2e-2 gate). PE ~226us is the design bottleneck.
  - exp is split: each slot's two score matmuls write separate [128,512]
    PSUM tiles (ring of 4); per half-tile a fixed 32-period pattern
    assigns ACT (exact exp, 75%) or a DVE 2-term Schraudolph (25%):
    ia = trunc(score*A + B) int16 straight from PSUM, ib = ia - 512,
    pt = f16bits(ia) * f16bits(ib); the pair product cancels most of
    the linear-interp error (rms ~0.5%). Strict A/S interleaving keeps
    the exp engines off each other's ring slots. (GPSIMD cannot touch
    PSUM on real hw, so Pool only gets SBUF-side work: obs normalize,
    em scaling, memsets.)
  - ctx accumulated as in v1: [128 q, 65] chains packed into 2 PSUM
    banks, trailing cursor on a 16-deep pt ring; projection/boundary
    drains are emitted ~2 slots late so they never head-block the
    in-order DVE queue on a still-running PE chain.
"""

import sys
from contextlib import ExitStack

import numpy as np

sys.path.insert(0, "/opt/trn_rl_repo")

import concourse.bass as bass  # noqa: E402
from concourse import bacc  # noqa: E402
import concourse.mybir as mybir  # noqa: E402
import concourse.tile as tile  # noqa: E402

B, S, H = 4, 2048, 1024
NH, HD = 16, 64
GH = 8            # heads per core
GC = GH * HD      # 512 output cols per core
NP = 128          # partitions
NQ = S // 512     # 4 q blocks of 512
NKT = S // NP     # 16 k tiles of 128
NST = 16
F32 = mybir.dt.float32
F16 = mybir.dt.float16
I16 = mybir.dt.int16
F8E4 = mybir.dt.float8e4
F8E5 = mybir.dt.float8e5
SCALE = 1.0 / 8.0  # 1/sqrt(HD)
EXPF = mybir.ActivationFunctionType.Exp
DR = mybir.MatmulPerfMode.DoubleRow

# 2-term Schraudolph constants: ia = trunc(score * A2S + B2) (int16),
# exp(score/8) ~= f16bits(ia) * f16bits(ia - 512).
import math as _math
A2S = float(SCALE * 512.0 / _math.log(2.0))
B2 = 15561.25
WSCL = 1.0 / 16.0  # W host-prescale compensation in drains

import os as _os
N_WARM = int(_os.environ.get("K_NWARM", 11))
PT_RING = int(_os.environ.get("K_PTRING", 16))
CTX_LAG = int(_os.environ.get("K_CTXLAG", 14))
DRIP_NS = int(_os.environ.get("K_DRIP", 250))
BND_GATE = int(_os.environ.get("K_BGATE", 3))
# per-half exp engine pattern, period 32 over half-index (2*pos + i):
# True = DVE 2-term Schraudolph (8/32), False = exact ACT exp. Stride-4
# placement on odd halves keeps ACT/DVE strictly interleaved on the
# 4-deep PSUM score ring.
_S32S = {
    "8": {1, 5, 9, 13, 17, 21, 25, 29},
    "9o": {1, 3, 5, 9, 13, 17, 21, 25, 29},
    "9b": {1, 5, 9, 11, 15, 19, 23, 27, 31},
    "8p3": {3, 7, 11, 15, 19, 23, 27, 31},
    "10o": {1, 3, 7, 11, 13, 17, 21, 23, 27, 31},
    "9c": {1, 5, 7, 11, 15, 19, 23, 27, 31},
    "9d": {1, 5, 9, 13, 15, 19, 23, 27, 31},
    "9e": {3, 5, 9, 13, 17, 19, 23, 27, 31},
    "9f": {1, 5, 9, 13, 15, 19, 23, 27, 29},
    "9g": {1, 3, 7, 11, 15, 19, 23, 27, 31},
    "9h": {1, 5, 9, 13, 15, 17, 21, 25, 29},
}
_S32 = _S32S[_os.environ.get("K_S32", "9d")]
SCH_PAT = tuple(i in _S32 for i in range(32))


def _emit(tc, x8, x8r, wq8, wq8r, wk8, wk8r, wv8, wv8r, bqd, bkd, bvd,
          maskd, out):
    import heapq

    nc = tc.nc
    with ExitStack() as ctx:
        const = ctx.enter_context(tc.tile_pool(name="const", bufs=1))
        big = ctx.enter_context(tc.tile_pool(name="big", bufs=1))

        # ---- consts ----
        mask_sb = const.tile([NP, NKT], F32, tag="mask")
        em = const.tile([NP, NKT], F32, tag="em")
        bk_sb = const.tile([NP, 4], F32, tag="bk")
        bq_sb = const.tile([NP, 4], F32, tag="bq")
        bv_bc = const.tile([NP, GC], F32, tag="bvbc")

        # persistent SBUF tensors
        x8_sb = big.tile([NP, 16384], F8E4, tag="x8", name="x8")
        x8r_sb = big.tile([NP, 16384], F8E5, tag="x8r", name="x8r")
        qt_sb = [big.tile([NP, S], F16, tag=f"qt{i}", name=f"qt{i}") for i in range(4)]
        kt_sb = [big.tile([NP, S], F16, tag=f"kt{i}", name=f"kt{i}") for i in range(4)]
        v_sb = [
            [
                big.tile([NP, 2 * (HD + 1)], F16, tag=f"v{p}_{i}", name=f"v{p}_{i}")
                for i in range(NST)
            ]
            for p in range(4)
        ]
        wq_sb = big.tile([NP, 4096], F8E4, tag="wq")
        wqr_sb = big.tile([NP, 4096], F8E5, tag="wqr")
        wk_sb = big.tile([NP, 4096], F8E4, tag="wk")
        wkr_sb = big.tile([NP, 4096], F8E5, tag="wkr")
        wv_sb = big.tile([NP, 4096], F8E4, tag="wv")
        wvr_sb = big.tile([NP, 4096], F8E5, tag="wvr")

        def ld(dst, src, c0, cw):
            nc.sync.dma_start(out=dst[:, c0:c0 + cw], in_=src[:, c0:c0 + cw])

        # ---- DMA queue, ordered by first use; each DMACopy holds SP.SEQ
        # ~650ns, so the fat pr0-chain inputs issue FIRST and the tiny
        # bias/mask transfers (needed only at drain time ~5.5us) follow ----
        ld(wk_sb, wk8, 0, 1024)    # mt0 slices first: pr0 K/Q chains
        ld(wq_sb, wq8, 0, 1024)
        ld(x8_sb, x8, 0, 4096)     # s-block 0
        ld(wkr_sb, wk8r, 0, 1024)
        ld(wqr_sb, wq8r, 0, 1024)
        ld(x8r_sb, x8r, 0, 4096)
        nc.sync.dma_start(out=bk_sb[:], in_=bkd.rearrange("(m p) -> p m", p=NP))
        nc.sync.dma_start(out=bq_sb[:], in_=bqd.rearrange("(m p) -> p m", p=NP))
        nc.sync.dma_start(out=mask_sb[:], in_=maskd.rearrange("(t p) -> p t", p=NP))
        nc.scalar.activation(em[:], mask_sb[:], EXPF)  # warms Exp table too
        for sb in range(1, 4):     # s-blocks 1-3 (scores kt=4sb need them)
            ld(x8_sb, x8, sb * 4096, 4096)
            ld(x8r_sb, x8r, sb * 4096, 4096)
        ld(wv_sb, wv8, 0, 4096)
        ld(wvr_sb, wv8r, 0, 4096)
        nc.sync.dma_start(out=bv_bc[:], in_=bvd)
        ld(wk_sb, wk8, 1024, 3072)
        ld(wq_sb, wq8, 1024, 3072)
        ld(wkr_sb, wk8r, 1024, 3072)
        ld(wqr_sb, wq8r, 1024, 3072)

        def w_ap(w, mt, j):
            base = (mt * 4 + j) * 256
            return w[:, base:base + 256].rearrange("p (i m) -> p i m", i=2)

        def x_ap(xsb, sblk, j):
            base = (sblk * 4 + j) * 1024
            return xsb[:, base:base + 1024].rearrange("p (i s) -> p i s", i=2)

        with (
            tc.tile_pool(name="psS", bufs=4, space="PSUM") as psS,
            tc.tile_pool(name="psC", bufs=1, space="PSUM") as psC,
            tc.tile_pool(name="psB", bufs=2, space="PSUM") as psB,
            tc.tile_pool(name="ptpool", bufs=PT_RING) as ptpool,
            tc.tile_pool(name="iapool", bufs=3) as iapool,
            tc.tile_pool(name="ibpool", bufs=3) as ibpool,
            tc.tile_pool(name="cspool", bufs=4) as cspool,
            tc.tile_pool(name="rcpool", bufs=4) as rcpool,
            tc.tile_pool(name="obpool", bufs=1) as obpool,
        ):
            ctxps = [
                psC.tile([NP, 512], F32, tag=f"ctx{i}", name=f"ctx{i}")
                for i in range(2)
            ]
            obs = {
                qt: obpool.tile([NP, 4 * GC], F16, tag=f"ob{qt}", name="ob")
                for qt in range(NQ)
            }

            v_done = set()
            CUR = [0]       # current stream slot (for deferred drains)

            # terms: (w8,x8), (w8r,x8), (w8,x8r) - x8r arrives last via DMA
            _terms = ((0, 0), (1, 0), (0, 1))

            def v_chain(pr, st):
                """Project V cols for head pair pr, s-tile st (fp8 3-term)."""
                cell = {}
                sblk, sw = divmod(st, 4)

                def mm(t, j, cell=cell):
                    if t == 0 and j == 0:
                        cell["pv"] = psB.tile(
                            [NP, 512], F32, tag="proj", name="pv"
                        )[:, 0:NP]
                    wt, xt_ = _terms[t]
                    w = (wv_sb, wvr_sb)[wt]
                    xx = (x8_sb, x8r_sb)[xt_]
                    nc.tensor.matmul(
                        cell["pv"],
                        x_ap(xx, sblk, j)[:, :, sw * NP:(sw + 1) * NP],
                        w_ap(w, pr, j),
                        start=(t == 0 and j == 0),
                        stop=(t == 2 and j == 3),
                        perf_mode=DR,
                    )

                def drain(cell=cell):
                    v3 = v_sb[pr][st][:].rearrange("p (h e) -> p h e", e=HD + 1)
                    nc.gpsimd.memset(v3[:, :, HD], 1.0)
                    nc.vector.scalar_tensor_tensor(
                        out=v3[:, :, 0:HD],
                        in0=cell["pv"].rearrange("p (h e) -> p h e", e=HD),
                        scalar=WSCL,
                        in1=bv_bc[:, pr * NP:(pr + 1) * NP].rearrange(
                            "p (h e) -> p h e", e=HD
                        ),
                        op0=mybir.AluOpType.mult,
                        op1=mybir.AluOpType.add,
                    )
                    nc.gpsimd.tensor_scalar_mul(
                        v_sb[pr][st][:], v_sb[pr][st][:], em[:, st:st + 1]
                    )
                    v_done.add((pr, st))

                def lastmm(mm=mm, drain=drain):
                    # defer the drain ~2 slots so it never head-blocks the
                    # DVE queue waiting on this chain's PE completion
                    mm(2, 3)
                    push(CUR[0] + 2, [(0, drain)])

                return [
                    (27, lambda t=t, j=j, mm=mm: mm(t, j))
                    for t in range(3) for j in range(4)
                ][:-1] + [(27, lastmm)]

            def qk_chain(mt, which, nt, defer=True):
                wm, wr, dst, bias = (
                    (wk_sb, wkr_sb, kt_sb, bk_sb),
                    (wq_sb, wqr_sb, qt_sb, bq_sb),
                )[which]
                cell = {}

                def mm(t, j, cell=cell):
                    if t == 0 and j == 0:
                        cell["pp"] = psB.tile([NP, 512], F32, tag="proj", name="pp")
                    wt, xt_ = _terms[t]
                    w = (wm, wr)[wt]
                    xx = (x8_sb, x8r_sb)[xt_]
                    nc.tensor.matmul(
                        cell["pp"][:],
                        w_ap(w, mt, j),
                        x_ap(xx, nt, j),
                        start=(t == 0 and j == 0),
                        stop=(t == 2 and j == 3),
                        perf_mode=DR,
                    )

                def drain(cell=cell):
                    nc.vector.tensor_scalar(
                        out=dst[mt][:, nt * 512:(nt + 1) * 512],
                        in0=cell["pp"][:],
                        scalar1=WSCL,
                        scalar2=bias[:, mt:mt + 1],
                        op0=mybir.AluOpType.mult,
                        op1=mybir.AluOpType.add,
                    )

                def lastmm(mm=mm, drain=drain):
                    mm(2, 3)
                    push(CUR[0] + 2, [(0, drain)])

                units = [
                    (107, lambda t=t, j=j, mm=mm: mm(t, j))
                    for t in range(3) for j in range(4)
                ]
                if defer:
                    return units[:-1] + [(107, lastmm)]
                return units + [(0, drain)]

            # ---- prologue: warm-up matmuls keep the PE p-state ramping
            # through the DMA-bound head; then the pr0 nt0 K/Q chains ----
            wz = const.tile([NP, 512], F16, tag="wz")
            nc.gpsimd.memset(wz[:], 0.0)
            wps = psB.tile([NP, 512], F32, tag="proj", name="wps")
            for _ in range(N_WARM):
                nc.tensor.matmul(wps[:], wz[:, 0:NP], wz[:], start=True, stop=True)
            for _, u in qk_chain(0, 0, 0, defer=False) + qk_chain(0, 1, 0, defer=False):
                u()

            # ---- filler backlog, prioritized by need-slot ----
            backlog = []
            bseq = [0]

            def push(need, units):
                heapq.heappush(backlog, [need, bseq[0], units, [0]])
                bseq[0] += 1

            ventries = {}

            def pushv(need, p, st):
                entry = [need, bseq[0], v_chain(p, st), [0]]
                bseq[0] += 1
                ventries[(p, st)] = entry
                heapq.heappush(backlog, entry)

            # pr0: K nt1-3 land just after their x8/x8r chunks; Q nt per its
            # first consumer slot; V once wv/wvr are in (~slot 13)
            for nt, kn in ((1, 1), (2, 4), (3, 8)):
                push(kn, qk_chain(0, 0, nt))
            for nt, qn in ((1, 8), (2, 22), (3, 38)):
                push(qn, qk_chain(0, 1, nt))
            for st in range(NST):
                pushv(14 + st * 5 // 4, 0, st)
            for p in (1, 2, 3):
                push(64 * p - 22, qk_chain(p, 0, 0))
                push(64 * p - 14, qk_chain(p, 1, 0))
                for j in (1, 2, 3):
                    push(64 * p + 4 * j - 4, qk_chain(p, 0, j))
                for nt in (1, 2, 3):
                    push(64 * p + 16 * nt - 10, qk_chain(p, 1, nt))
                for st in range(NST):
                    pushv(64 * p - 24 + st, p, st)

            def pop_entry_units(entry, budget):
                need, seq, units, idx = entry
                spent = 0
                while idx[0] < len(units):
                    cost, u = units[idx[0]]
                    if spent > 0 and spent + cost > budget:
                        break
                    u()
                    spent += cost
                    idx[0] += 1
                return spent, idx[0] >= len(units)

            def force(need):
                while backlog and backlog[0][0] <= need:
                    entry = heapq.heappop(backlog)
                    pop_entry_units(entry, 10 ** 9)

            def drip(budget):
                while backlog and budget > 0:
                    # pop BEFORE running: units may push new entries (deferred
                    # drains) with lower need than this one
                    entry = heapq.heappop(backlog)
                    spent, done = pop_entry_units(entry, budget)
                    budget -= spent
                    if not done:
                        heapq.heappush(backlog, entry)
                        break

            # ---- ctx cursor ----
            pt_slots = {}
            ctx_c = [0]

            def emit_boundary(cq):
                # stage 1: drain the ctx PSUM chains (DVE); stage 2 (obs
                # normalize + stores) is deferred 2 more slots so the Pool
                # muls / store DMAs never head-block their queues waiting
                # on stage 1
                pr, qt = divmod(cq, 4)
                css, rcs = [], []
                for i in range(2):
                    cs = cspool.tile([NP, 260], F32, tag="cs", name="cs")
                    nc.vector.tensor_copy(cs[:], ctxps[i][:, 0:260])
                    rc = rcpool.tile([NP, 4], F32, tag="rc", name="rc")
                    nc.vector.reciprocal(
                        rc[:],
                        cs[:].rearrange("p (j e) -> p j e", e=HD + 1)[:, :, HD],
                    )
                    css.append(cs)
                    rcs.append(rc)
                push(CUR[0] + 2, [(0, lambda: emit_obs(cq, css, rcs))])

            def emit_obs(cq, css, rcs):
                pr, qt = divmod(cq, 4)
                oeng = nc.vector if cq >= 14 else nc.gpsimd
                for j in range(4):
                    for i in range(2):
                        hh = 2 * pr + i
                        oeng.tensor_scalar_mul(
                            obs[qt][:, j * GC + hh * HD:j * GC + (hh + 1) * HD],
                            css[i][:, 65 * j:65 * j + HD],
                            rcs[i][:, j:j + 1],
                        )
                    if pr == 3:
                        jq = qt * 4 + j
                        eng = nc.scalar if (qt == 3 and j % 2 == 1) else nc.sync
                        eng.dma_start(
                            out=out[jq * NP:(jq + 1) * NP, :],
                            in_=obs[qt][:, j * GC:(j + 1) * GC],
                        )

            bnd_emitted = {}

            def emit_ctx_group(c):
                cq, ck = divmod(c, NKT)
                pr = cq // 4
                pt = pt_slots.pop(c)
                for i in range(2):
                    for j in range(4):
                        nc.tensor.matmul(
                            ctxps[i][:, 65 * j:65 * j + 65],
                            pt[:, i * 512 + j * NP:i * 512 + (j + 1) * NP],
                            v_sb[pr][ck][:, i * 65:(i + 1) * 65],
                            start=(ck == 0 and j == 0),
                            stop=(ck == NKT - 1),
                            skip_group_check=True,
                        )
                if ck == NKT - 1:
                    # defer the boundary drain ~2 slots: its cs copies wait
                    # on this chain's last matmuls; emitting late keeps the
                    # DVE queue head unblocked
                    def bnd(cq=cq):
                        bnd_emitted[cq] = CUR[0]
                        emit_boundary(cq)

                    push(CUR[0] + 2, [(0, bnd)])

            def ctx_due(pos):
                c = ctx_c[0]
                if c >= min(pos, 256):
                    return False
                cq, ck = divmod(c, NKT)
                if ck == 0 and cq > 0:
                    # ctxps reuse: wait until the previous boundary's cs
                    # copies are emitted and have cleared the DVE queue
                    if cq - 1 not in bnd_emitted or pos < bnd_emitted[cq - 1] + BND_GATE:
                        return False
                return (cq // 4, ck) in v_done

            # ---- the global stream ----
            pos = 0
            for pr in range(4):
                for qt in range(NQ):
                    for kt in range(NKT):
                        CUR[0] = pos
                        force(pos)
                        pt = ptpool.tile([NP, 1024], F16, tag="pt", name="pt")
                        for i in range(2):
                            sc = psS.tile([NP, 512], F32, tag="sc", name="sc")
                            nc.tensor.matmul(
                                sc[:],
                                kt_sb[pr][i * 64:(i + 1) * 64,
                                          kt * NP:(kt + 1) * NP],
                                qt_sb[pr][i * 64:(i + 1) * 64,
                                          qt * 512:(qt + 1) * 512],
                                start=True,
                                stop=True,
                                tile_position=(i * 64, 0),
                            )
                            half = pt[:, i * 512:(i + 1) * 512]
                            if SCH_PAT[(2 * pos + i) % len(SCH_PAT)] and pos >= int(_os.environ.get('K_SMIN', 4)):
                                ia = iapool.tile([NP, 512], I16, tag="ia",
                                                 name="ia")
                                nc.vector.tensor_scalar(
                                    out=ia[:], in0=sc[:], scalar1=A2S,
                                    scalar2=B2, op0=mybir.AluOpType.mult,
                                    op1=mybir.AluOpType.add,
                                )
                                ib = ibpool.tile([NP, 512], I16, tag="ib",
                                                 name="ib")
                                nc.vector.tensor_scalar(
                                    out=ib[:], in0=ia[:], scalar1=512,
                                    scalar2=None, op0=mybir.AluOpType.subtract,
                                )
                                nc.vector.tensor_tensor(
                                    out=half,
                                    in0=ia[:].bitcast(F16),
                                    in1=ib[:].bitcast(F16),
                                    op=mybir.AluOpType.mult,
                                )
                            else:
                                nc.scalar.activation(half, sc[:], EXPF,
                                                     scale=SCALE)
                        pt_slots[pos] = pt
                        pos += 1
                        n_ctx = 2 + (pos > 150) + (pos > 210)
                        emitted = 0
                        while emitted < n_ctx and ctx_due(pos):
                            emit_ctx_group(ctx_c[0])
                            ctx_c[0] += 1
                            emitted += 1
                        # fillers rationed so the backlog lasts the whole
                        # stream (total filler ~62us / 256 slots)
                        drip(DRIP_NS)
                        while pos - ctx_c[0] >= CTX_LAG:
                            cq, ck = divmod(ctx_c[0], NKT)
                            ve = ventries.get((cq // 4, ck))
                            if ve is not None:
                                force(ve[0])
                            if not ctx_due(pos):
                                force(pos + 2)  # pull deferred drains
                            if not ctx_due(pos):
                                break
                            emit_ctx_group(ctx_c[0])
                            ctx_c[0] += 1
            # tail: flush whatever is left
            CUR[0] = 260
            while ctx_c[0] < 256:
                force(10 ** 9)
                assert ctx_due(10 ** 9), f"ctx stuck at {ctx_c[0]}"
                emit_ctx_group(ctx_c[0])
                ctx_c[0] += 1
            force(10 ** 9)


_NC_CACHE = {}


def _get_nc():
    if "nc" not in _NC_CACHE:
        nc = bacc.Bacc("TRN2", target_bir_lowering=False, debug=False,
                       enable_asserts=False)
        x8 = nc.dram_tensor("x8", [NP, 16384], F8E4, kind="ExternalInput").ap()
        x8r = nc.dram_tensor("x8r", [NP, 16384], F8E5, kind="ExternalInput").ap()
        wq8 = nc.dram_tensor("wq8", [NP, 4096], F8E4, kind="ExternalInput").ap()
        wq8r = nc.dram_tensor("wq8r", [NP, 4096], F8E5, kind="ExternalInput").ap()
        wk8 = nc.dram_tensor("wk8", [NP, 4096], F8E4, kind="ExternalInput").ap()
        wk8r = nc.dram_tensor("wk8r", [NP, 4096], F8E5, kind="ExternalInput").ap()
        wv8 = nc.dram_tensor("wv8", [NP, 4096], F8E4, kind="ExternalInput").ap()
        wv8r = nc.dram_tensor("wv8r", [NP, 4096], F8E5, kind="ExternalInput").ap()
        bq = nc.dram_tensor("bq", [GC], F32, kind="ExternalInput").ap()
        bk = nc.dram_tensor("bk", [GC], F32, kind="ExternalInput").ap()
        bv = nc.dram_tensor("bv", [NP, GC], F32, kind="ExternalInput").ap()
        mask = nc.dram_tensor("mask", [S], F32, kind="ExternalInput").ap()
        out = nc.dram_tensor("out", [S, GC], F16, kind="ExternalOutput").ap()
        with tile.TileContext(nc) as tc:
            _emit(tc, x8, x8r, wq8, wq8r, wk8, wk8r, wv8, wv8r, bq, bk, bv,
                  mask, out)
        nc.compile()
        _NC_CACHE["nc"] = nc
    return _NC_CACHE["nc"]


def _pack_x(x16):
    """[2048 s, 1024 c] -> [128 p, (sblk j i s512)] fp8 pair (e4m3, e5m2)."""
    from ml_dtypes import float8_e4m3, float8_e5m2

    x8 = x16.astype(float8_e4m3)
    x8r = (x16.astype(np.float32) - x8.astype(np.float32)).astype(float8_e5m2)

    def lay(a):
        # c = 256j + 128i + p ; s = 512*sblk + s'
        t = np.ascontiguousarray(a.T)              # [1024 c, 2048 s]
        t = t.reshape(4, 2, 128, 4, 512)           # [j, i, p, sblk, s']
        t = t.transpose(2, 3, 0, 1, 4)             # [p, sblk, j, i, s']
        return np.ascontiguousarray(t.reshape(128, 16384))

    return lay(x8), lay(x8r)


def _pack_w(w16):
    """[1024 c, 512 m] -> [128 p, (mt j i m)] fp8 pair, prescaled x16."""
    from ml_dtypes import float8_e4m3, float8_e5m2

    ws = w16.astype(np.float32) * 16.0
    w8 = ws.astype(float8_e4m3)
    w8r = (ws - w8.astype(np.float32)).astype(float8_e5m2)

    def lay(a):
        t = a.reshape(4, 2, 128, 4, 128)           # [j, i, p, mt, m]
        t = t.transpose(2, 3, 0, 1, 4)             # [p, mt, j, i, m]
        return np.ascontiguousarray(t.reshape(128, 4096))

    return lay(w8), lay(w8r)


def _in_maps(inputs):
    hs = np.asarray(inputs["hidden_states"], np.float32)
    am = np.asarray(inputs["attention_mask"], np.float32)
    ws = {k: np.asarray(inputs[k], np.float32) for k in ("Wq", "Wk", "Wv")}
    bs = {k: np.asarray(inputs[k], np.float32) for k in ("bq", "bk", "bv")}
    maps = []
    for c in range(8):
        b, g = c // 2, c % 2
        cols = slice(g * GC, (g + 1) * GC)
        x8, x8r = _pack_x(hs[b].astype(np.float16))
        m = {"x8": x8, "x8r": x8r}
        for nm, wn in (("q", "Wq"), ("k", "Wk"), ("v", "Wv")):
            w8, w8r = _pack_w(ws[wn][:, cols].astype(np.float16))
            m[f"w{nm}8"] = w8
            m[f"w{nm}8r"] = w8r
        m["bq"] = np.ascontiguousarray(bs["bq"][cols])
        m["bk"] = np.ascontiguousarray(bs["bk"][cols])
        m["bv"] = np.ascontiguousarray(np.broadcast_to(bs["bv"][cols], (NP, GC)))
        m["mask"] = np.ascontiguousarray(am[b, 0, 0, :])
        maps.append(m)
    return maps


class _Runner:
    """Cached PJRT executor for the SPMD bass program (8 cores)."""

    def __init__(self, nc, n_cores=8):
        import jax
        from jax.experimental.shard_map import shard_map
        from jax.sharding import Mesh, PartitionSpec

        from concourse import bass2jax, mybir as _mybir

        bass2jax.install_neuronx_cc_hook()
        self.jax = jax
        self.nc = nc
        self.n_cores = n_cores
        assert nc.dbg_addr is None
        part_name = (
            nc.partition_id_tensor.name if nc.partition_id_tensor is not None else None
        )

        in_names, out_names, out_avals, zero_outs = [], [], [], []
        for alloc in nc.m.functions[0].allocations:
            if not isinstance(alloc, _mybir.MemoryLocationSet):
                continue
            name = alloc.memorylocations[0].name
            if alloc.kind == "ExternalInput":
                if name != part_name:
                    in_names.append(name)
            elif alloc.kind == "ExternalOutput":
                out_names.append(name)
                shape = tuple(alloc.tensor_shape)
                dtype = _mybir.dt.np(alloc.dtype)
                out_avals.append(jax.core.ShapedArray(shape, dtype))
                zero_outs.append(np.zeros(shape, dtype))
        self.in_names = list(in_names)
        self.out_names = list(out_names)
        self.out_avals = out_avals
        self.zero_outs = zero_outs
        n_params, n_outs = len(in_names), len(out_avals)
        all_names = in_names + out_names
        if part_name is not None:
            all_names = all_names + [part_name]
        donate = tuple(range(n_params, n_params + n_outs))

        def _body(*args):
            operands = list(args)
            if part_name is not None:
                operands.append(bass2jax.partition_id_tensor())
            outs = bass2jax._bass_exec_p.bind(
                *operands,
                out_avals=tuple(out_avals),
                in_names=tuple(all_names),
                out_names=tuple(out_names),
                lowering_input_output_aliases=(),
                sim_require_finite=True,
                sim_require_nnan=True,
                nc=nc,
            )
            return tuple(outs)

        self._body = _body
        devices = jax.devices()[:n_cores]
        self.mesh = Mesh(np.asarray(devices), ("core",))
        self.pspec = PartitionSpec("core")
        in_specs = (self.pspec,) * (n_params + n_outs)
        out_specs = (self.pspec,) * n_outs
        self.sharded = jax.jit(
            shard_map(
                _body,
                mesh=self.mesh,
                in_specs=in_specs,
                out_specs=out_specs,
                check_rep=False,
            ),
            donate_argnums=donate,
            keep_unused=True,
        )

    def concat_inputs(self, in_maps):
        return [
            np.concatenate([np.asarray(m[name]) for m in in_maps], axis=0)
            for name in self.in_names
        ]

    def fresh_zeros(self):
        return [
            np.zeros((self.n_cores * z.shape[0], *z.shape[1:]), z.dtype)
            for z in self.zero_outs
        ]

    def __call__(self, in_maps):
        out_arrs = self.sharded(*self.concat_inputs(in_maps), *self.fresh_zeros())
        return [
            {
                name: np.asarray(out_arrs[i]).reshape(
                    self.n_cores, *self.out_avals[i].shape
                )[c]
                for i, name in enumerate(self.out_names)
            }
            for c in range(self.n_cores)
        ]


def _get_runner():
    if "runner" not in _NC_CACHE:
        _NC_CACHE["runner"] = _Runner(_get_nc())
    return _NC_CACHE["runner"]


def _assemble(results):
    full = np.empty((B, S, H), np.float32)
    for c in range(8):
        b, g = c // 2, c % 2
        full[b, :, g * GC:(g + 1) * GC] = results[c]["out"].astype(np.float32)
    return full


def _run(inputs, trace=False, **kwargs):
    if trace:
        from concourse.bass_utils import run_bass_kernel_spmd

        nc = _get_nc()
        res = run_bass_kernel_spmd(
            nc, _in_maps(inputs), core_ids=list(range(8)), trace=True, **kwargs
        )
        return _assemble(res.results), res

    return _assemble(_get_runner()(_in_maps(inputs))), None


def kernel(**inputs):
    return _run(inputs)[0]
